# revision 1
# baseline (speedup 1.0000x reference)
"""Phi3 decoder layer on 8 Trainium2 NeuronCores (tensor-parallel).

Sharding: qkv/gate_up column-sharded, o/down row-sharded over 8 cores
(4 q-heads + 1 kv-head per core). Activations kept transposed ([hid, s])
on device. Residual adds are folded as hidden/8 into the row-sharded
partial sums so the AllReduce/ReduceScatter outputs are the full
hs / final output directly. The sequence dim is processed in 4 tiles of
512 so the two collectives pipeline behind compute of later tiles.
"""
import math

import numpy as np
import ml_dtypes

import concourse.bass as bass
import concourse.tile as tile
import concourse.mybir as mybir
from concourse import bass_utils
from concourse.tile import ScopedClock

# ---------------------------------------------------------------- constants
B, S, HID = 1, 2048, 3072
NH, NKV, D = 32, 8, 96
INTER = 8192
EPS = 1e-5
NCORES = 8
QH = NH // NCORES            # 4 q heads per core
OPC = QH * D + 2 * D         # 576 qkv out cols per core
DMC = QH * D                 # 384 attn model dims per core (3 x 128)
GUC = 2 * INTER // NCORES    # 2048 gate_up cols per core (1024 g + 1024 u)
DIC = INTER // NCORES        # 1024 down rows per core (8 x 128)
HC = HID // 128              # 24 hid chunks
ST = 512                     # s tile
NST = S // ST                # 4
KC = 128                     # k chunk in attention
NKC = S // KC                # 16
SM_SCALE = 1.0 / math.sqrt(D)
NEG = -1e30

F32 = mybir.dt.float32
BF16 = mybir.dt.bfloat16
AF = mybir.ActivationFunctionType
ALU = mybir.AluOpType

# ------------------------------------------------------- walrus workarounds
# This walrus build encodes at most ONE sync wait per instruction. Tile's
# exit drain and any multi-producer instruction exceed that; split extra
# waits onto single-wait NoOps on the same (in-order) engine.
_split_counter = [0]


def _patched_drain_and_barrier(self, tick_clock, wait_clock):
    drain_inst = self.nc.sync.drain()
    wait_clock.add_sem_waits(
        drain_inst.ins, ScopedClock({None: tick_clock.global_clock})
    )
    si = drain_inst.ins.sync_info
    if si is not None and si.on_wait and len(si.on_wait) > 1:
        waits = list(si.on_wait)
        upd = list(si.on_update) if si.on_update else []
        drain_inst.ins.sync_info = mybir.SyncInfo(on_wait=[waits[0]], on_update=upd)
        for w in waits[1:]:
            n = self.nc.sync.nop()
            n.ins.sync_info = mybir.SyncInfo(on_wait=[w], on_update=[])
    self.nc.all_engine_barrier()
    assert self.sems is not None
    popped = self.nc._tile_sem_poison_stack.pop()
    assert popped is self._sem_poison
    self.nc.clear_and_free_semaphores(list(self.sems.allocated().values()))
    self.nc.all_engine_barrier()


def _split_multi_waits(nc):
    for fn in nc.m.functions:
        for bb in fn.blocks:
            insts = list(bb.instructions)
            out = []
            changed = False
            for inst in insts:
                si = inst.sync_info
                if si is not None and si.on_wait and len(si.on_wait) > 1:
                    waits = list(si.on_wait)
                    upd = list(si.on_update) if si.on_update else []
                    for w in waits[:-1]:
                        _split_counter[0] += 1
                        n = mybir.InstNoOp(
                            name=f"I-waitsplit-{_split_counter[0]}", ins=[], outs=[]
                        )
                        n.engine = inst.engine
                        n.sync_info = mybir.SyncInfo(on_wait=[w], on_update=[])
                        out.append(n)
                    inst.sync_info = mybir.SyncInfo(on_wait=[waits[-1]], on_update=upd)
                    changed = True
                out.append(inst)
            if changed:
                bb.instructions = out


tile.TileContext._drain_and_barrier = _patched_drain_and_barrier

# ------------------------------------------------------------- kernel build


def build_nc(attn_table, nbias):
    """attn_table[st] = list of (kchunk, bias_idx) with bias_idx=-1 for fully
    open blocks; nbias = number of bias patterns (>=1)."""
    nc = bass.Bass("TRN2", num_devices=NCORES)

    xT = nc.dram_tensor("xT", [HID, S], F32, kind="ExternalInput")
    wqkv = nc.dram_tensor("wqkv", [QH + 2, 128, HC * D], BF16, kind="ExternalInput")
    wo = nc.dram_tensor("wo", [HC, 128, 3 * 128], BF16, kind="ExternalInput")
    wgu_g = nc.dram_tensor("wgu_g", [DIC // 128, 128, HID], BF16, kind="ExternalInput")
    wgu_u = nc.dram_tensor("wgu_u", [DIC // 128, 128, HID], BF16, kind="ExternalInput")
    wd = nc.dram_tensor("wd", [HC, 128, DIC], BF16, kind="ExternalInput")
    sinT = nc.dram_tensor("sinT", [D, S], F32, kind="ExternalInput")
    cosT = nc.dram_tensor("cosT", [D, S], F32, kind="ExternalInput")
    ident_in = nc.dram_tensor("ident", [128, 128], BF16, kind="ExternalInput")
    pmat_in = nc.dram_tensor("pmat", [D, D], F32, kind="ExternalInput")
    biasp = nc.dram_tensor("biasp", [128, nbias, ST], BF16, kind="ExternalInput")
    out_shard = nc.dram_tensor("out_shard", [DMC, S], F32, kind="ExternalOutput")

    o_in = [nc.dram_tensor(f"o_in{st}", [HID, ST], BF16) for st in range(NST)]
    hs_sh = [
        nc.dram_tensor(f"hs_sh{st}", [HID, ST], BF16, addr_space="Shared")
        for st in range(NST)
    ]
    d_in = [nc.dram_tensor(f"d_in{st}", [HID, ST], BF16) for st in range(NST)]
    rs_o = [nc.dram_tensor(f"rs_o{st}", [DMC, ST], BF16) for st in range(NST)]
    rg = [list(range(NCORES))]

    with tile.TileContext(nc) as tc:
        with (
            tc.tile_pool(name="const", bufs=1) as consts,
            tc.tile_pool(name="big", bufs=1) as big,
            tc.tile_pool(name="wstream", bufs=4) as wpool,
            tc.tile_pool(name="work", bufs=2) as work,
            tc.tile_pool(name="psA", bufs=2, space="PSUM") as psA,
            tc.tile_pool(name="psB", bufs=2, space="PSUM") as psB,
            tc.tile_pool(name="psC", bufs=2, space="PSUM") as psC,
            tc.tile_pool(name="psD", bufs=2, space="PSUM") as psD,
        ):
            # ---------------- persistent constants
            sin_sb = consts.tile([D, S], F32, name="sin_sb")
            cos_sb = consts.tile([D, S], F32, name="cos_sb")
            nc.sync.dma_start(sin_sb[:], sinT.ap())
            nc.sync.dma_start(cos_sb[:], cosT.ap())
            ident = consts.tile([128, 128], BF16, name="ident")
            nc.sync.dma_start(ident[:], ident_in.ap())
            pmat = consts.tile([D, D], F32, name="pmat")
            nc.sync.dma_start(pmat[:], pmat_in.ap())
            bias_sb = consts.tile([128, nbias, ST], BF16, name="bias_sb")
            nc.sync.dma_start(bias_sb[:], biasp.ap())
            onesb = consts.tile([128, 1], BF16, name="onesb")
            nc.vector.memset(onesb[:], 1.0)
            ones1 = consts.tile([1, 128], F32, name="ones1")
            nc.vector.memset(ones1[:], 1.0)
            epsc = consts.tile([1, 1], F32, name="epsc")
            nc.vector.memset(epsc[:], EPS)
            KT = consts.tile([D, S], BF16, name="KT")
            Vk = consts.tile([128, NKC, D + 1], BF16, name="Vk")
            nc.vector.memset(Vk[:, :, D:D + 1], 1.0)

            def rmsnorm_to_bf16(src_dram, cols, xbf):
                """Read src_dram[:, cols] chunks, write normalized bf16 into
                xbf [128, HC, ST] (in-place two-pass)."""
                pss = psD.tile([1, ST], F32, name="pss", tag="psD")
                for g in range(HC // 6):
                    xin6 = work.tile([128, 6, ST], src_dram.dtype, name="xin6",
                                     tag="xin6", bufs=2)
                    src = src_dram.ap()[g * 6 * 128:(g + 1) * 6 * 128, cols]
                    nc.scalar.dma_start(
                        xin6[:], src.rearrange("(c p) s -> p c s", p=128)
                    )
                    for i in range(6):
                        hcx = g * 6 + i
                        xsq = work.tile([128, ST], BF16, name="xsq", tag="xsq")
                        nc.scalar.activation(xsq[:], xin6[:, i, :], AF.Square)
                        nc.tensor.matmul(
                            pss[:], onesb[:], xsq[:],
                            start=(hcx == 0), stop=(hcx == HC - 1),
                        )
                        nc.vector.tensor_copy(xbf[:, hcx, :], xin6[:, i, :])
                sstd = work.tile([1, ST], F32, name="sstd", tag="sstd")
                nc.scalar.activation(sstd[:], pss[:], AF.Sqrt,
                                     scale=1.0 / HID, bias=epsc[0:1, 0:1])
                rstd = work.tile([1, ST], F32, name="rstd", tag="sstd")
                nc.vector.reciprocal(rstd[:], sstd[:])
                pbc = psD.tile([128, ST], F32, name="pbc", tag="psD")
                nc.tensor.matmul(pbc[:], ones1[:], rstd[:], start=True, stop=True)
                rbc = work.tile([128, ST], F32, name="rbc", tag="rbc")
                nc.scalar.copy(rbc[:], pbc[:])
                for hcx in range(HC):
                    nc.vector.tensor_mul(xbf[:, hcx, :], xbf[:, hcx, :], rbc[:])

            def rope(dst, qsb, st):
                """dst [D, ST] bf16 <- rope(qsb [D, ST] f32 sbuf) at s-tile st.
                rotate_half is a signed 96x96 permutation done on the PE."""
                sl = slice(st * ST, (st + 1) * ST)
                prot = psD.tile([D, ST], F32, name="prot", tag="psD")
                nc.tensor.matmul(prot[:], pmat[:], qsb[:], start=True, stop=True)
                tcs = work.tile([D, ST], BF16, name="tcs", tag="ropec")
                nc.vector.tensor_mul(tcs[:], qsb[:], cos_sb[:, sl])
                trs = work.tile([D, ST], BF16, name="trs", tag="ropes")
                nc.vector.tensor_mul(trs[:], prot[:], sin_sb[:, sl])
                nc.vector.tensor_add(dst, tcs[:], trs[:])

            def phaseABC(st):
                ssl = slice(st * ST, (st + 1) * ST)
                # ============ phase A: rmsnorm1 + qkv + rope =============
                xbf = big.tile([128, HC, ST], BF16, name="xbf", tag="xbf")
                rmsnorm_to_bf16(xT, ssl, xbf)

                QT = work.tile([D, QH, ST], BF16, name="QT", tag="QT")
                for m in range(QH + 2):
                    pq = psA.tile([D, ST], F32, name="pq", tag="psA")
                    wqm = wpool.tile([128, HC, D], BF16, name="wqm", tag="wqm",
                                     bufs=2)
                    nc.sync.dma_start(
                        wqm[:], wqkv.ap()[m].rearrange("p (hc o) -> p hc o", o=D)
                    )
                    for hcx in range(HC):
                        nc.tensor.matmul(
                            pq[:], wqm[:, hcx, :], xbf[:, hcx, :],
                            start=(hcx == 0), stop=(hcx == HC - 1),
                        )
                    if m < QH:
                        qsb = work.tile([D, ST], F32, name="qsb", tag="qsb")
                        nc.scalar.copy(qsb[:], pq[:])
                        rope(QT[:, m, :], qsb, st)
                    elif m == QH:
                        qsb = work.tile([D, ST], F32, name="qsb", tag="qsb")
                        nc.scalar.copy(qsb[:], pq[:])
                        rope(KT[:, ssl], qsb, st)
                    else:
                        vt = work.tile([D, ST], BF16, name="vt", tag="vt")
                        nc.scalar.copy(vt[:], pq[:])
                        for c4 in range(ST // 128):
                            ptr = psD.tile([128, D], BF16, name="ptr", tag="psD")
                            nc.tensor.transpose(
                                ptr[:], vt[:, c4 * 128:(c4 + 1) * 128],
                                ident[0:D, 0:D],
                            )
                            nc.vector.tensor_copy(
                                Vk[:, st * (ST // 128) + c4, 0:D], ptr[:]
                            )

                # ============ phase B: attention for q-tile st ===========
                a3 = [
                    work.tile([128, ST], BF16, name=f"a3_{j}", tag=f"a3_{j}")
                    for j in range(3)
                ]
                blocks = attn_table[st]
                for h in range(QH):
                    pa = psC.tile([D + 1, ST], F32, name="pa", tag="psC")
                    for bi, (kc, bidx) in enumerate(blocks):
                        ps = psB.tile([128, ST], F32, name="ps", tag="psB")
                        nc.tensor.matmul(
                            ps[:], KT[:, kc * KC:(kc + 1) * KC],
                            QT[:, h, :], start=True, stop=True,
                        )
                        probs = work.tile([128, ST], BF16, name="probs", tag="probs", bufs=3)
                        if bidx >= 0:
                            nc.vector.scalar_tensor_tensor(
                                ps[:], ps[:], SM_SCALE, bias_sb[:, bidx, :],
                                op0=ALU.mult, op1=ALU.add,
                            )
                            nc.scalar.activation(probs[:], ps[:], AF.Exp)
                        else:
                            nc.scalar.activation(probs[:], ps[:], AF.Exp,
                                                 scale=SM_SCALE)
                        nc.tensor.matmul(
                            pa[:], Vk[:, kc, :], probs[:],
                            start=(bi == 0), stop=(bi == len(blocks) - 1),
                        )
                    rec = work.tile([1, ST], F32, name="rec", tag="rec")
                    nc.vector.reciprocal(rec[:], pa[D:D + 1, :])
                    pbc2 = psD.tile([D, ST], F32, name="pbc2", tag="psD")
                    nc.tensor.matmul(pbc2[:], ones1[:, 0:D], rec[:],
                                     start=True, stop=True)
                    bcs = work.tile([D, ST], F32, name="bcs", tag="bcs")
                    nc.scalar.copy(bcs[:], pbc2[:])
                    # scatter h-th head rows (96h..96h+96) into 128-row tiles
                    r0 = h * D
                    r1 = r0 + D
                    j0, j1 = r0 // 128, (r1 - 1) // 128
                    for j in range(j0, j1 + 1):
                        lo = max(r0, j * 128)
                        hi = min(r1, (j + 1) * 128)
                        # partition-offset accesses may span at most 32
                        # partitions unless they start at 0 -> 32-row pieces
                        for p0 in range(lo, hi, 32):
                            p1 = min(p0 + 32, hi)
                            nc.vector.tensor_mul(
                                a3[j][p0 - j * 128:p1 - j * 128, :],
                                pa[p0 - r0:p1 - r0, :],
                                bcs[p0 - r0:p1 - r0, :],
                            )

                # ============ phase C: o-proj partial + hidden/8 + AR ====
                for m in range(HC):
                    if m % 6 == 0:
                        xr6 = work.tile([128, 6, ST], F32, name="xr6", tag="xin6",
                                        bufs=2)
                        src = xT.ap()[m * 128:(m + 6) * 128, ssl]
                        nc.scalar.dma_start(
                            xr6[:], src.rearrange("(c p) s -> p c s", p=128)
                        )
                    po = psA.tile([128, ST], F32, name="po", tag="psA")
                    wom = wpool.tile([128, 3, 128], BF16, name="wom", tag="wom",
                                     bufs=3)
                    nc.sync.dma_start(
                        wom[:], wo.ap()[m].rearrange("p (j o) -> p j o", o=128)
                    )
                    for j in range(3):
                        nc.tensor.matmul(
                            po[:], wom[:, j, :], a3[j][:],
                            start=(j == 0), stop=(j == 2),
                        )
                    ob = work.tile([128, ST], BF16, name="ob", tag="ob", bufs=3)
                    nc.vector.scalar_tensor_tensor(
                        ob[:], xr6[:, m % 6, :], 1.0 / NCORES, po[:],
                        op0=ALU.mult, op1=ALU.add,
                    )
                    nc.scalar.dma_start(
                        o_in[st].ap()[m * 128:(m + 1) * 128, :], ob[:]
                    )
                nc.gpsimd.collective_compute(
                    "AllReduce", ALU.add, replica_groups=rg,
                    ins=[o_in[st].ap().opt()], outs=[hs_sh[st].ap().opt()],
                )

            def phaseDEF(st):
                ssl = slice(st * ST, (st + 1) * ST)
                # ============ phase D: rmsnorm2 ==========================
                hbf = big.tile([128, HC, ST], BF16, name="hbf", tag="hbf")
                rmsnorm_to_bf16(hs_sh[st], slice(0, ST), hbf)

                # ============ phase E: gate_up + silu*up =================
                act = big.tile([128, DIC // 128, ST], BF16, name="act", tag="act")
                for gm in range(DIC // 128):
                    pg = psA.tile([128, ST], F32, name="pg", tag="psA")
                    pu = psB.tile([128, ST], F32, name="pu", tag="psB")
                    wgt = wpool.tile([128, HC, 128], BF16, name="wgt", tag="wgt",
                                     bufs=2)
                    nc.sync.dma_start(
                        wgt[:], wgu_g.ap()[gm].rearrange("p (hc o) -> p hc o",
                                                         o=128))
                    wut = wpool.tile([128, HC, 128], BF16, name="wut", tag="wut",
                                     bufs=2)
                    nc.sync.dma_start(
                        wut[:], wgu_u.ap()[gm].rearrange("p (hc o) -> p hc o",
                                                         o=128))
                    for hcx in range(HC):
                        nc.tensor.matmul(pg[:], wgt[:, hcx, :], hbf[:, hcx, :],
                                         start=(hcx == 0), stop=(hcx == HC - 1))
                    for hcx in range(HC):
                        nc.tensor.matmul(pu[:], wut[:, hcx, :], hbf[:, hcx, :],
                                         start=(hcx == 0), stop=(hcx == HC - 1))
                    sg = work.tile([128, ST], F32, name="sg", tag="sg")
                    nc.scalar.activation(sg[:], pg[:], AF.Silu)
                    nc.vector.tensor_mul(act[:, gm, :], sg[:], pu[:])

                # ============ phase F: down + hs/8 + RS + out ============
                for m in range(HC):
                    if m % 6 == 0:
                        hr6 = work.tile([128, 6, ST], BF16, name="hr6",
                                        tag="xin6", bufs=2)
                        src = hs_sh[st].ap()[m * 128:(m + 6) * 128, :]
                        nc.scalar.dma_start(
                            hr6[:], src.rearrange("(c p) s -> p c s", p=128)
                        )
                    pd = psA.tile([128, ST], F32, name="pd", tag="psA")
                    wdm = wpool.tile([128, DIC // 128, 128], BF16, name="wdm",
                                     tag="wdm", bufs=3)
                    nc.sync.dma_start(
                        wdm[:], wd.ap()[m].rearrange("p (ic o) -> p ic o", o=128)
                    )
                    for ic in range(DIC // 128):
                        nc.tensor.matmul(
                            pd[:], wdm[:, ic, :], act[:, ic, :],
                            start=(ic == 0), stop=(ic == DIC // 128 - 1),
                        )
                    db = work.tile([128, ST], BF16, name="db", tag="ob", bufs=3)
                    nc.vector.scalar_tensor_tensor(
                        db[:], hr6[:, m % 6, :], 1.0 / NCORES, pd[:],
                        op0=ALU.mult, op1=ALU.add,
                    )
                    nc.scalar.dma_start(
                        d_in[st].ap()[m * 128:(m + 1) * 128, :], db[:]
                    )
                nc.gpsimd.collective_compute(
                    "ReduceScatter", ALU.add, replica_groups=rg,
                    ins=[d_in[st].ap().opt()], outs=[rs_o[st].ap().opt()],
                )
                for j in range(DMC // 128):
                    oshard = work.tile([128, ST], BF16, name="oshard", tag="ob",
                                       bufs=3)
                    nc.sync.dma_start(
                        oshard[:], rs_o[st].ap()[j * 128:(j + 1) * 128, :]
                    )
                    osf = work.tile([128, ST], F32, name="osf", tag="osf", bufs=2)
                    nc.vector.tensor_copy(osf[:], oshard[:])
                    nc.sync.dma_start(
                        out_shard.ap()[j * 128:(j + 1) * 128, ssl], osf[:]
                    )

            # software pipeline: AR(st) completes while ABC(st+1) computes;
            # RS(st) completes while ABC(st+2)/DEF(st+1) compute.
            phaseABC(0)
            phaseABC(1)
            for st in range(2, NST):
                phaseDEF(st - 2)
                phaseABC(st)
            phaseDEF(NST - 2)
            phaseDEF(NST - 1)

    _split_multi_waits(nc)
    return nc


# --------------------------------------------------------------- host side
_NC_CACHE = {}


def _get_nc(table_key, attn_table, nbias):
    if table_key not in _NC_CACHE:
        _NC_CACHE[table_key] = build_nc(attn_table, nbias)
    return _NC_CACHE[table_key]


def kernel(hidden_states, sin, cos, attention_mask, position_ids,
           qkv_kernel, o_kernel, gate_up_kernel, down_kernel, ln1_w, ln2_w):
    hidden_states = np.asarray(hidden_states)
    sin = np.asarray(sin)
    cos = np.asarray(cos)
    attention_mask = np.asarray(attention_mask)
    position_ids = np.asarray(position_ids)
    qkv_kernel = np.asarray(qkv_kernel, np.float32)
    o_kernel = np.asarray(o_kernel, np.float32)
    gate_up_kernel = np.asarray(gate_up_kernel, np.float32)
    down_kernel = np.asarray(down_kernel, np.float32)
    ln1_w = np.asarray(ln1_w, np.float32)
    ln2_w = np.asarray(ln2_w, np.float32)

    bf = ml_dtypes.bfloat16
    # mask -> per-block classification (q-tile 512 x k-chunk 128)
    mask = np.asarray(attention_mask[0, 0])  # [S(q), S(k)]
    patterns = {}
    pat_arrays = []
    attn_table = []
    for st in range(NST):
        rows = []
        sub_q = mask[st * ST:(st + 1) * ST, :]
        for kc in range(NKC):
            blk = sub_q[:, kc * KC:(kc + 1) * KC]  # [512 q, 128 k]
            if blk.min() > 0:
                rows.append((kc, -1))
            elif blk.max() <= 0:
                continue
            else:
                bt = np.where(blk.T > 0, np.float32(0.0),
                              np.float32(NEG)).astype(bf)  # [128 k, 512 q]
                key = bt.tobytes()
                if key not in patterns:
                    patterns[key] = len(pat_arrays)
                    pat_arrays.append(bt)
                rows.append((kc, patterns[key]))
        attn_table.append(tuple(rows))
    nbias = max(1, len(pat_arrays))
    if not pat_arrays:
        pat_arrays = [np.zeros((KC, ST), bf)]
    biasp = np.stack(pat_arrays, axis=1)  # [128, nbias, 512]

    table_key = (tuple(attn_table), nbias)
    nc = _get_nc(table_key, attn_table, nbias)

    # transposed activations + rope tables gathered by position_ids
    xT = np.ascontiguousarray(hidden_states[0].T.astype(np.float32))  # [HID, S]
    pos = np.asarray(position_ids[0])
    sinT = np.ascontiguousarray(np.asarray(sin)[pos].T.astype(np.float32))
    cosT = np.ascontiguousarray(np.asarray(cos)[pos].T.astype(np.float32))
    ident = np.eye(128, dtype=bf)
    P = np.zeros((D, D), np.float32)
    for i in range(D // 2):
        P[i, i + D // 2] = -1.0
        P[i + D // 2, i] = 1.0
    pmat = np.ascontiguousarray(P.T)

    # fold ln weights into the column-sharded projections
    wqkv_full = (qkv_kernel * ln1_w[:, None]).astype(bf)    # [HID, OP]
    wgu_full = (gate_up_kernel * ln2_w[:, None]).astype(bf)  # [HID, 2*INTER]
    wo_full = o_kernel.astype(bf)                            # [HID, HID]
    wd_full = down_kernel.astype(bf)                         # [INTER, HID]

    in_maps = []
    for c in range(NCORES):
        qcols = wqkv_full[:, c * QH * D:(c + 1) * QH * D]
        kcols = wqkv_full[:, NH * D + c * D:NH * D + (c + 1) * D]
        vcols = wqkv_full[:, NH * D + NKV * D + c * D:
                          NH * D + NKV * D + (c + 1) * D]
        wqkv_c = np.concatenate([qcols, kcols, vcols], 1)      # [HID, OPC]
        # [m, p, hc*D]: tile m holds W[hc*128+p, m*D+o] at [p, hc*D+o]
        wqkv_t = np.ascontiguousarray(
            wqkv_c.reshape(HC, 128, QH + 2, D).transpose(2, 1, 0, 3)
            .reshape(QH + 2, 128, HC * D))
        wo_c = wo_full[c * DMC:(c + 1) * DMC, :]               # [384, HID]
        wo_t = np.ascontiguousarray(
            wo_c.reshape(3, 128, HC, 128).transpose(2, 1, 0, 3)
            .reshape(HC, 128, 3 * 128))
        gslice = wgu_full[:, c * DIC:(c + 1) * DIC]            # [HID, 1024]
        uslice = wgu_full[:, INTER + c * DIC:INTER + (c + 1) * DIC]
        wgu_gt = np.ascontiguousarray(
            gslice.reshape(HC, 128, DIC // 128, 128).transpose(2, 1, 0, 3)
            .reshape(DIC // 128, 128, HID))
        wgu_ut = np.ascontiguousarray(
            uslice.reshape(HC, 128, DIC // 128, 128).transpose(2, 1, 0, 3)
            .reshape(DIC // 128, 128, HID))
        wd_c = wd_full[c * DIC:(c + 1) * DIC, :]               # [1024, HID]
        wd_t = np.ascontiguousarray(
            wd_c.reshape(DIC // 128, 128, HC, 128).transpose(2, 1, 0, 3)
            .reshape(HC, 128, DIC))
        in_maps.append(dict(
            xT=xT, wqkv=wqkv_t, wo=wo_t, wgu_g=wgu_gt, wgu_u=wgu_ut, wd=wd_t,
            sinT=sinT, cosT=cosT, ident=ident, pmat=pmat, biasp=biasp,
        ))

    res = bass_utils.run_bass_kernel_spmd(nc, in_maps,
                                          core_ids=list(range(NCORES)))
    outT = np.concatenate([res.results[c]["out_shard"] for c in range(NCORES)],
                          axis=0)  # [HID, S]
    return np.ascontiguousarray(outT.T)[None].astype(np.float32)



# revision 2
# speedup vs baseline: 1.0060x; 1.0060x over previous
"""Phi3 decoder layer on 8 Trainium2 NeuronCores (tensor-parallel).

vs v1: rmsnorm1 on host (xn bf16 uploaded); AllGather of attn heads +
hs shards (0.39MB/rank) instead of 3.1MB AllReduce; rmsnorm2 deferred to
gate_up outputs (matmuls never wait on the norm); single activation
table set (exp/ln); reciprocal_approx_fast; paired QK blocks with
batched exp; residual returned as a second output and added on host.
"""
import math

import numpy as np
import ml_dtypes

import concourse.bass as bass
import concourse.tile as tile
import concourse.mybir as mybir
from concourse import bass_utils
from concourse.tile import ScopedClock

# ---------------------------------------------------------------- constants
B, S, HID = 1, 2048, 3072
NH, NKV, D = 32, 8, 96
INTER = 8192
EPS = 1e-5
NCORES = 8
QH = NH // NCORES            # 4 q heads per core
DMC = QH * D                 # 384 hid rows per core (3 x 128)
DIC = INTER // NCORES        # 1024 inter rows per core (8 x 128)
HC = HID // 128              # 24 hid chunks
ST = 512                     # s tile
NST = S // ST                # 4
KC = 128                     # k chunk in attention
NKC = S // KC                # 16
SM_SCALE = 1.0 / math.sqrt(D)
NEG = -1e30
AGR = DMC + 1                # 385 rows per rank in the hs AllGather

F32 = mybir.dt.float32
BF16 = mybir.dt.bfloat16
AF = mybir.ActivationFunctionType
ALU = mybir.AluOpType

# ------------------------------------------------------- walrus workarounds
# This walrus build encodes at most ONE sync wait per instruction. Tile's
# exit drain and any multi-producer instruction exceed that; split extra
# waits onto single-wait NoOps on the same (in-order) engine.
_split_counter = [0]


def _patched_drain_and_barrier(self, tick_clock, wait_clock):
    drain_inst = self.nc.sync.drain()
    wait_clock.add_sem_waits(
        drain_inst.ins, ScopedClock({None: tick_clock.global_clock})
    )
    si = drain_inst.ins.sync_info
    if si is not None and si.on_wait and len(si.on_wait) > 1:
        waits = list(si.on_wait)
        upd = list(si.on_update) if si.on_update else []
        drain_inst.ins.sync_info = mybir.SyncInfo(on_wait=[waits[0]], on_update=upd)
        for w in waits[1:]:
            n = self.nc.sync.nop()
            n.ins.sync_info = mybir.SyncInfo(on_wait=[w], on_update=[])
    self.nc.all_engine_barrier()
    assert self.sems is not None
    popped = self.nc._tile_sem_poison_stack.pop()
    assert popped is self._sem_poison
    self.nc.clear_and_free_semaphores(list(self.sems.allocated().values()))
    self.nc.all_engine_barrier()


def _split_multi_waits(nc):
    for fn in nc.m.functions:
        for bb in fn.blocks:
            insts = list(bb.instructions)
            out = []
            changed = False
            for inst in insts:
                si = inst.sync_info
                if si is not None and si.on_wait and len(si.on_wait) > 1:
                    waits = list(si.on_wait)
                    upd = list(si.on_update) if si.on_update else []
                    for w in waits[:-1]:
                        _split_counter[0] += 1
                        n = mybir.InstNoOp(
                            name=f"I-waitsplit-{_split_counter[0]}", ins=[], outs=[]
                        )
                        n.engine = inst.engine
                        n.sync_info = mybir.SyncInfo(on_wait=[w], on_update=[])
                        out.append(n)
                    inst.sync_info = mybir.SyncInfo(on_wait=[waits[-1]], on_update=upd)
                    changed = True
                out.append(inst)
            if changed:
                bb.instructions = out


tile.TileContext._drain_and_barrier = _patched_drain_and_barrier

# ------------------------------------------------------------- kernel build


def build_nc(attn_table, nbias):
    """attn_table[st] = list of (kchunk, bias_idx) with bias_idx=-1 for fully
    open blocks; nbias = number of bias patterns (>=1)."""
    nc = bass.Bass("TRN2", num_devices=NCORES)

    xnT = nc.dram_tensor("xnT", [HID, S], BF16, kind="ExternalInput")
    xres = nc.dram_tensor("xres", [3, 128, S], BF16, kind="ExternalInput")
    wqkv = nc.dram_tensor("wqkv", [QH + 2, 128, HC * D], BF16, kind="ExternalInput")
    wo = nc.dram_tensor("wo", [3, 128, HC * 128], BF16, kind="ExternalInput")
    wgu_g = nc.dram_tensor("wgu_g", [DIC // 128, 128, HID], BF16, kind="ExternalInput")
    wgu_u = nc.dram_tensor("wgu_u", [DIC // 128, 128, HID], BF16, kind="ExternalInput")
    wd = nc.dram_tensor("wd", [HC, 128, DIC], BF16, kind="ExternalInput")
    sinT = nc.dram_tensor("sinT", [D, S], BF16, kind="ExternalInput")
    cosT = nc.dram_tensor("cosT", [D, S], BF16, kind="ExternalInput")
    ident_in = nc.dram_tensor("ident", [128, 128], BF16, kind="ExternalInput")
    pmat_in = nc.dram_tensor("pmat", [D, D], BF16, kind="ExternalInput")
    biasp = nc.dram_tensor("biasp", [128, nbias, ST], BF16, kind="ExternalInput")
    mlp_shard = nc.dram_tensor("mlp_shard", [DMC, S], BF16, kind="ExternalOutput")
    hs_shard = nc.dram_tensor("hs_shard", [DMC, S], BF16, kind="ExternalOutput")

    att_in = [nc.dram_tensor(f"att_in{st}", [DMC, ST], BF16) for st in range(NST)]
    att_out = [
        nc.dram_tensor(f"att_out{st}", [HID, ST], BF16, addr_space="Shared")
        for st in range(NST)
    ]
    hsg_in = [nc.dram_tensor(f"hsg_in{st}", [AGR, ST], BF16) for st in range(NST)]
    hsg_out = [
        nc.dram_tensor(f"hsg_out{st}", [NCORES * AGR, ST], BF16, addr_space="Shared")
        for st in range(NST)
    ]
    d_in = [nc.dram_tensor(f"d_in{st}", [HID, ST], BF16) for st in range(NST)]
    rs_o = [nc.dram_tensor(f"rs_o{st}", [DMC, ST], BF16) for st in range(NST)]
    rg = [list(range(NCORES))]

    with tile.TileContext(nc) as tc:
        with (
            tc.tile_pool(name="const", bufs=1) as consts,
            tc.tile_pool(name="wstream", bufs=1) as wpool,
            tc.tile_pool(name="work", bufs=1) as work,
            tc.tile_pool(name="psBig", bufs=2, space="PSUM") as psBig,
            tc.tile_pool(name="psPV", bufs=2, space="PSUM") as psPV,
            tc.tile_pool(name="psS", bufs=2, space="PSUM") as psS,
        ):
            # ---------------- persistent constants
            sin_sb = consts.tile([D, S], BF16, name="sin_sb")
            cos_sb = consts.tile([D, S], BF16, name="cos_sb")
            nc.gpsimd.dma_start(sin_sb[:], sinT.ap())
            nc.gpsimd.dma_start(cos_sb[:], cosT.ap())
            ident = consts.tile([128, 128], BF16, name="ident")
            nc.gpsimd.dma_start(ident[:], ident_in.ap())
            pmat = consts.tile([D, D], BF16, name="pmat")
            nc.gpsimd.dma_start(pmat[:], pmat_in.ap())
            bias_sb = consts.tile([128, nbias, ST], BF16, name="bias_sb")
            nc.gpsimd.dma_start(bias_sb[:], biasp.ap())
            onesb = consts.tile([128, 1], BF16, name="onesb")
            nc.vector.memset(onesb[:], 1.0)
            ones8 = consts.tile([8, 1], BF16, name="ones8")
            nc.vector.memset(ones8[:], 1.0)
            ones1f = consts.tile([1, 128], F32, name="ones1f")
            nc.vector.memset(ones1f[:], 1.0)
            ones1b = consts.tile([1, 128], BF16, name="ones1b")
            nc.vector.memset(ones1b[:], 1.0)
            epsc = consts.tile([1, 1], F32, name="epsc")
            nc.vector.memset(epsc[:], EPS)
            KT = consts.tile([D, S], BF16, name="KT")
            Vk = consts.tile([128, NKC, D + 1], BF16, name="Vk")
            nc.vector.memset(Vk[:, :, D:D + 1], 1.0)

            # a3 scatter pieces per head: (j, dst_lo, dst_hi, src_lo)
            # pieces >32 partitions are only legal when BOTH src (pa row
            # p0-r0) and dst (a3 row p0-128j) offsets are 0
            a3_pieces = []
            for h in range(QH):
                r0, r1 = h * D, (h + 1) * D
                pieces = []
                for j in range(r0 // 128, (r1 - 1) // 128 + 1):
                    lo, hi = max(r0, j * 128), min(r1, (j + 1) * 128)
                    if lo == j * 128 and lo == r0:
                        pieces.append((j, 0, hi - lo, 0))
                    else:
                        for p0 in range(lo, hi, 32):
                            p1 = min(p0 + 32, hi)
                            pieces.append((j, p0 - j * 128, p1 - j * 128, p0 - r0))
                a3_pieces.append(pieces)

            def rope(dst, qsb, st):
                """dst [D, ST] bf16 <- rope(qsb [D, ST] bf16) at s-tile st."""
                sl = slice(st * ST, (st + 1) * ST)
                prot = psS.tile([D, ST], F32, name="prot", tag="psS")
                nc.tensor.matmul(prot[:], pmat[:], qsb[:], start=True, stop=True)
                tcs = work.tile([D, ST], BF16, name="tcs", tag="ropec")
                nc.vector.tensor_mul(tcs[:], qsb[:], cos_sb[:, sl])
                trs = work.tile([D, ST], BF16, name="trs", tag="ropes")
                nc.vector.tensor_mul(trs[:], prot[:], sin_sb[:, sl])
                nc.vector.tensor_add(dst, tcs[:], trs[:])

            def phaseA(st):
                ssl = slice(st * ST, (st + 1) * ST)
                xh = []
                for half in range(2):
                    xt = work.tile([128, HC // 2, ST], BF16, name=f"xh{half}",
                                   tag="xbf", bufs=2)
                    src = xnT.ap()[half * 12 * 128:(half + 1) * 12 * 128, ssl]
                    nc.scalar.dma_start(
                        xt[:], src.rearrange("(c p) s -> p c s", p=128)
                    )
                    xh.append(xt)
                QT = work.tile([D, QH, ST], BF16, name="QT", tag="QT")
                for m in range(QH + 2):
                    pq = psPV.tile([D, ST], F32, name="pq", tag="psPV")
                    wqm = wpool.tile([128, HC, D], BF16, name="wqm", tag="wqm",
                                     bufs=2)
                    nc.sync.dma_start(
                        wqm[:], wqkv.ap()[m].rearrange("p (hc o) -> p hc o", o=D)
                    )
                    for hcx in range(HC):
                        nc.tensor.matmul(
                            pq[:], wqm[:, hcx, :], xh[hcx // 12][:, hcx % 12, :],
                            start=(hcx == 0), stop=(hcx == HC - 1),
                        )
                    qsb = work.tile([D, ST], BF16, name="qsb", tag="qsb", bufs=2)
                    nc.scalar.copy(qsb[:], pq[:])
                    if m < QH:
                        rope(QT[:, m, :], qsb, st)
                    elif m == QH:
                        rope(KT[:, ssl], qsb, st)
                    else:
                        for c4 in range(ST // 128):
                            ptr = psS.tile([128, D], BF16, name="ptr", tag="psS")
                            nc.tensor.transpose(
                                ptr[:], qsb[:, c4 * 128:(c4 + 1) * 128],
                                ident[0:D, 0:D],
                            )
                            nc.vector.tensor_copy(
                                Vk[:, st * (ST // 128) + c4, 0:D], ptr[:]
                            )

                # -------- attention (phase B)
                a3 = work.tile([128, 3, ST], BF16, name="a3", tag="a3")
                blocks = attn_table[st]
                pairs = [blocks[i:i + 2] for i in range(0, len(blocks), 2)]
                for h in range(QH):
                    pa = psPV.tile([D + 1, ST], F32, name="pa", tag="psPV")
                    nmm = 0
                    for pi, pr in enumerate(pairs):
                        w = len(pr)
                        psq = psBig.tile([128, 2, ST], F32, name="psq", tag="psBig")
                        for i, (kc, bidx) in enumerate(pr):
                            nc.tensor.matmul(
                                psq[:, i, :], KT[:, kc * KC:(kc + 1) * KC],
                                QT[:, h, :], start=True, stop=True,
                            )
                        bidxs = [bidx for (kc, bidx) in pr]
                        probs = work.tile([128, 2, ST], BF16, name="probs",
                                          tag="probs", bufs=2)
                        nc.scalar.activation(probs[:, 0:w, :], psq[:, 0:w, :],
                                             AF.Exp)
                        if w == 2 and bidxs[0] >= 0 and bidxs[1] == bidxs[0] + 1:
                            nc.vector.tensor_mul(
                                probs[:, 0:2, :], probs[:, 0:2, :],
                                bias_sb[:, bidxs[0]:bidxs[0] + 2, :],
                            )
                        else:
                            for i, (kc, bidx) in enumerate(pr):
                                if bidx >= 0:
                                    nc.vector.tensor_mul(
                                        probs[:, i, :], probs[:, i, :],
                                        bias_sb[:, bidx, :],
                                    )
                        for i, (kc, bidx) in enumerate(pr):
                            nmm += 1
                            nc.tensor.matmul(
                                pa[:], Vk[:, kc, :], probs[:, i, :],
                                start=(nmm == 1), stop=(nmm == len(blocks)),
                            )
                    rec = work.tile([1, ST], F32, name="rec", tag="rec", bufs=2)
                    nc.vector.reciprocal_approx_fast(rec[:], pa[D:D + 1, :])
                    pbc2 = psS.tile([D, ST], F32, name="pbc2", tag="psS")
                    nc.tensor.matmul(pbc2[:], ones1f[:, 0:D], rec[:],
                                     start=True, stop=True)
                    bcs = work.tile([D, ST], F32, name="bcs", tag="bcs")
                    nc.vector.tensor_copy(bcs[:], pbc2[:])
                    for (j, d0, d1, s0) in a3_pieces[h]:
                        nc.vector.tensor_mul(
                            a3[d0:d1, j, :],
                            pa[s0:s0 + (d1 - d0), :],
                            bcs[s0:s0 + (d1 - d0), :],
                        )
                # -------- phase C: ship heads, AllGather
                nc.gpsimd.dma_start(
                    att_in[st].ap().rearrange("(j p) s -> p j s", p=128), a3[:]
                )
                nc.gpsimd.collective_compute(
                    "AllGather", ALU.bypass, replica_groups=rg,
                    ins=[att_in[st].ap().opt()], outs=[att_out[st].ap().opt()],
                )

            def phaseD(st):
                ssl = slice(st * ST, (st + 1) * ST)
                ath = []
                for half in range(2):
                    at = work.tile([128, HC // 2, ST], BF16, name=f"at{half}",
                                   tag="attf", bufs=2)
                    src = att_out[st].ap()[half * 12 * 128:(half + 1) * 12 * 128, :]
                    nc.scalar.dma_start(
                        at[:], src.rearrange("(c p) s -> p c s", p=128)
                    )
                    ath.append(at)
                xr = work.tile([128, 3, ST], BF16, name="xr", tag="xres")
                nc.scalar.dma_start(
                    xr[:], xres.ap()[:, :, ssl].rearrange("j p s -> p j s")
                )
                ob = work.tile([128, 3, ST], BF16, name="ob", tag="ob", bufs=2)
                pssq = psS.tile([1, ST], F32, name="pssq", tag="psS")
                for j in range(3):
                    po = psPV.tile([128, ST], F32, name="po", tag="psPV")
                    woj = wpool.tile([128, HC, 128], BF16, name="woj", tag="wo",
                                     bufs=2)
                    nc.sync.dma_start(
                        woj[:], wo.ap()[j].rearrange("p (hc o) -> p hc o", o=128)
                    )
                    for hcx in range(HC):
                        nc.tensor.matmul(
                            po[:], woj[:, hcx, :], ath[hcx // 12][:, hcx % 12, :],
                            start=(hcx == 0), stop=(hcx == HC - 1),
                        )
                    nc.vector.tensor_add(ob[:, j, :], xr[:, j, :], po[:])
                    sq = work.tile([128, ST], BF16, name="sq", tag="sq")
                    nc.vector.tensor_mul(sq[:], ob[:, j, :], ob[:, j, :])
                    nc.tensor.matmul(pssq[:], onesb[:], sq[:],
                                     start=(j == 0), stop=(j == 2))
                ssqb = work.tile([1, ST], BF16, name="ssqb", tag="ssqb", bufs=2)
                nc.vector.tensor_copy(ssqb[:], pssq[:])
                nc.gpsimd.dma_start(
                    hsg_in[st].ap()[0:DMC, :].rearrange("(j p) s -> p j s", p=128),
                    ob[:],
                )
                nc.gpsimd.dma_start(hsg_in[st].ap()[DMC:AGR, :], ssqb[:])
                nc.scalar.dma_start(
                    hs_shard.ap()[:, ssl].rearrange("(j p) s -> p j s", p=128),
                    ob[:],
                )
                nc.gpsimd.collective_compute(
                    "AllGather", ALU.bypass, replica_groups=rg,
                    ins=[hsg_in[st].ap().opt()], outs=[hsg_out[st].ap().opt()],
                )

            def phaseE(st):
                # rstd2 from the gathered per-core partial sum-squares
                ss8 = work.tile([8, 1, ST], BF16, name="ss8", tag="ss8")
                src = hsg_out[st].ap().rearrange("(c r) s -> c r s", r=AGR)
                nc.scalar.dma_start(ss8[:], src[:, DMC:AGR, :])
                psls = psS.tile([1, ST], F32, name="psls", tag="psS")
                nc.tensor.matmul(psls[:], ones8[:], ss8[:, 0, :],
                                 start=True, stop=True)
                lnv = work.tile([1, ST], F32, name="lnv", tag="lnv")
                nc.scalar.activation(lnv[:], psls[:], AF.Ln,
                                     scale=1.0 / HID, bias=epsc[0:1, 0:1])
                rstd = work.tile([1, ST], BF16, name="rstd", tag="rstd")
                nc.scalar.activation(rstd[:], lnv[:], AF.Exp, scale=-0.5)
                pbc = psS.tile([128, ST], F32, name="pbc", tag="psS")
                nc.tensor.matmul(pbc[:], ones1b[:], rstd[:], start=True, stop=True)
                rbc = work.tile([128, ST], F32, name="rbc", tag="rbc")
                nc.vector.tensor_copy(rbc[:], pbc[:])

                hh = []
                for half in range(2):
                    ht = work.tile([128, 4, 3, ST], BF16, name=f"hh{half}",
                                   tag="hbf", bufs=2)
                    for cl in range(4):
                        sh = src[4 * half + cl, 0:DMC, :]
                        nc.scalar.dma_start(
                            ht[:, cl, :, :],
                            sh.rearrange("(j p) s -> p j s", p=128),
                        )
                    hh.append(ht)
                act = work.tile([128, DIC // 128, ST], BF16, name="act", tag="act")
                for gm in range(DIC // 128):
                    wg = wpool.tile([128, HC, 128], BF16, name="wg", tag="wg",
                                    bufs=2)
                    nc.sync.dma_start(
                        wg[:], wgu_g.ap()[gm].rearrange("p (hc o) -> p hc o",
                                                        o=128))
                    wu = wpool.tile([128, HC, 128], BF16, name="wu", tag="wu",
                                    bufs=2)
                    nc.sync.dma_start(
                        wu[:], wgu_u.ap()[gm].rearrange("p (hc o) -> p hc o",
                                                        o=128))
                    pguv = psBig.tile([128, 2, ST], F32, name="pguv", tag="psBig")
                    for hcx in range(HC):
                        nc.tensor.matmul(
                            pguv[:, 0, :], wg[:, hcx, :],
                            hh[hcx // 12][:, (hcx // 3) % 4, hcx % 3, :],
                            start=(hcx == 0), stop=(hcx == HC - 1),
                        )
                    for hcx in range(HC):
                        nc.tensor.matmul(
                            pguv[:, 1, :], wu[:, hcx, :],
                            hh[hcx // 12][:, (hcx // 3) % 4, hcx % 3, :],
                            start=(hcx == 0), stop=(hcx == HC - 1),
                        )
                    G = work.tile([128, ST], F32, name="G", tag="G", bufs=2)
                    nc.vector.tensor_mul(G[:], pguv[:, 0, :], rbc[:])
                    e = work.tile([128, ST], F32, name="e", tag="e", bufs=2)
                    nc.scalar.activation(e[:], G[:], AF.Exp, scale=-1.0)
                    nc.vector.tensor_scalar_add(e[:], e[:], 1.0)
                    r = work.tile([128, ST], F32, name="r", tag="r")
                    nc.vector.reciprocal_approx_fast(r[:], e[:])
                    nc.vector.tensor_mul(G[:], G[:], r[:])
                    nc.vector.tensor_mul(e[:], pguv[:, 1, :], rbc[:])
                    nc.vector.tensor_mul(act[:, gm, :], G[:], e[:])
                return act

            def phaseF(st, act):
                db = None
                for m in range(HC):
                    wdp = wpool.tile([128, DIC], BF16, name="wdp",
                                     tag="wd", bufs=2)
                    nc.sync.dma_start(wdp[:], wd.ap()[m])
                    pd = psPV.tile([128, ST], F32, name="pd", tag="psPV")
                    for ic in range(DIC // 128):
                        nc.tensor.matmul(
                            pd[:], wdp[:, ic * 128:(ic + 1) * 128],
                            act[:, ic, :],
                            start=(ic == 0), stop=(ic == DIC // 128 - 1),
                        )
                    if m % 3 == 0:
                        db = work.tile([128, 3, ST], BF16, name="db", tag="db",
                                       bufs=3)
                    nc.vector.tensor_copy(db[:, m % 3, :], pd[:])
                    if m % 3 == 2:
                        dst = d_in[st].ap()[(m - 2) * 128:(m + 1) * 128, :]
                        nc.sync.dma_start(
                            dst.rearrange("(j p) s -> p j s", p=128), db[:]
                        )
                nc.gpsimd.collective_compute(
                    "ReduceScatter", ALU.add, replica_groups=rg,
                    ins=[d_in[st].ap().opt()], outs=[rs_o[st].ap().opt()],
                )

            def out_copy(st):
                ssl = slice(st * ST, (st + 1) * ST)
                rsb = work.tile([128, 3, ST], BF16, name="rsb", tag="db", bufs=3)
                nc.scalar.dma_start(
                    rsb[:], rs_o[st].ap().rearrange("(j p) s -> p j s", p=128)
                )
                nc.sync.dma_start(
                    mlp_shard.ap()[:, ssl].rearrange("(j p) s -> p j s", p=128),
                    rsb[:],
                )

            # software pipeline: AGs/RS of tile st complete while other
            # tiles' matmuls keep the PE dense.
            phaseA(0)
            phaseA(1)
            phaseA(2)
            phaseD(0)
            phaseA(3)
            act0 = phaseE(0)
            phaseD(1)
            phaseF(0, act0)
            act1 = phaseE(1)
            phaseD(2)
            phaseF(1, act1)
            act2 = phaseE(2)
            phaseD(3)
            out_copy(0)
            phaseF(2, act2)
            act3 = phaseE(3)
            out_copy(1)
            phaseF(3, act3)
            out_copy(2)
            out_copy(3)

    _split_multi_waits(nc)
    return nc


# --------------------------------------------------------------- host side
_NC_CACHE = {}


def _get_nc(table_key, attn_table, nbias):
    if table_key not in _NC_CACHE:
        _NC_CACHE[table_key] = build_nc(attn_table, nbias)
    return _NC_CACHE[table_key]


def kernel(hidden_states, sin, cos, attention_mask, position_ids,
           qkv_kernel, o_kernel, gate_up_kernel, down_kernel, ln1_w, ln2_w):
    hidden_states = np.asarray(hidden_states)
    sin = np.asarray(sin)
    cos = np.asarray(cos)
    attention_mask = np.asarray(attention_mask)
    position_ids = np.asarray(position_ids)
    qkv_kernel = np.asarray(qkv_kernel, np.float32)
    o_kernel = np.asarray(o_kernel, np.float32)
    gate_up_kernel = np.asarray(gate_up_kernel, np.float32)
    down_kernel = np.asarray(down_kernel, np.float32)
    ln1_w = np.asarray(ln1_w, np.float32)
    ln2_w = np.asarray(ln2_w, np.float32)

    bf = ml_dtypes.bfloat16
    # mask -> per-block classification (q-tile 512 x k-chunk 128)
    mask = np.asarray(attention_mask[0, 0])  # [S(q), S(k)]
    patterns = {}
    pat_arrays = []
    attn_table = []
    for st in range(NST):
        rows = []
        sub_q = mask[st * ST:(st + 1) * ST, :]
        for kc in range(NKC):
            blk = sub_q[:, kc * KC:(kc + 1) * KC]  # [512 q, 128 k]
            if blk.min() > 0:
                rows.append((kc, -1))
            elif blk.max() <= 0:
                continue
            else:
                bt = np.where(blk.T > 0, np.float32(1.0),
                              np.float32(0.0)).astype(bf)  # [128 k, 512 q]
                key = bt.tobytes()
                if key not in patterns:
                    patterns[key] = len(pat_arrays)
                    pat_arrays.append(bt)
                rows.append((kc, patterns[key]))
        attn_table.append(tuple(rows))
    nbias = max(1, len(pat_arrays))
    if not pat_arrays:
        pat_arrays = [np.zeros((KC, ST), bf)]
    biasp = np.stack(pat_arrays, axis=1)  # [128, nbias, 512]

    table_key = (tuple(attn_table), nbias)
    nc = _get_nc(table_key, attn_table, nbias)

    # host-side rmsnorm1 (fp32); partition-major tiled bf16 layouts so every
    # device DMA moves >=3KB-contiguous per partition
    x = hidden_states[0].astype(np.float32)                    # [S, HID]
    var = np.mean(np.square(x), axis=-1, keepdims=True)
    xn = (ln1_w[None, :] * (x / np.sqrt(var + EPS)))
    # [st, half, p, c*ST+s] <- xn.T[(half*12+c)*128+p, st*ST+s]
    xnT = np.ascontiguousarray(
        xn.T.reshape(2, HC // 2, 128, NST, ST).transpose(3, 0, 2, 1, 4)
        .reshape(NST, 2, 128, (HC // 2) * ST)).astype(bf)
    xT_bf = np.ascontiguousarray(x.T).astype(bf)               # [HID, S]

    pos = np.asarray(position_ids[0])
    sinT = np.ascontiguousarray(np.asarray(sin)[pos].T).astype(bf)
    cosT = np.ascontiguousarray(np.asarray(cos)[pos].T).astype(bf)
    ident = np.eye(128, dtype=bf)
    P = np.zeros((D, D), np.float32)
    for i in range(D // 2):
        P[i, i + D // 2] = -1.0
        P[i + D // 2, i] = 1.0
    pmat = np.ascontiguousarray(P.T).astype(bf)

    # weights: sm_scale folded into q columns; ln2 folded into gate_up
    wqkv_full = qkv_kernel.copy()
    wqkv_full[:, :NH * D] *= SM_SCALE
    wqkv_full = wqkv_full.astype(bf)                           # [HID, OP]
    wgu_full = (gate_up_kernel * ln2_w[:, None]).astype(bf)    # [HID, 2*INTER]
    wo_full = o_kernel.astype(bf)                              # [HID, HID]
    wd_full = down_kernel.astype(bf)                           # [INTER, HID]

    in_maps = []
    for c in range(NCORES):
        qcols = wqkv_full[:, c * QH * D:(c + 1) * QH * D]
        kcols = wqkv_full[:, NH * D + c * D:NH * D + (c + 1) * D]
        vcols = wqkv_full[:, NH * D + NKV * D + c * D:
                          NH * D + NKV * D + (c + 1) * D]
        wqkv_c = np.concatenate([qcols, kcols, vcols], 1)      # [HID, 576]
        wqkv_t = np.ascontiguousarray(
            wqkv_c.reshape(HC, 128, QH + 2, D).transpose(2, 1, 0, 3)
            .reshape(QH + 2, 128, HC * D))
        # o column-shard: [3, 128, HC*128], [j,p,hc*128+o] = wo[hc*128+p, c*384+j*128+o]
        wo_c = wo_full[:, c * DMC:(c + 1) * DMC]               # [HID, 384]
        wo_t = np.ascontiguousarray(
            wo_c.reshape(HC, 128, 3, 128).transpose(2, 1, 0, 3)
            .reshape(3, 128, HC * 128))
        gslice = wgu_full[:, c * DIC:(c + 1) * DIC]            # [HID, 1024]
        uslice = wgu_full[:, INTER + c * DIC:INTER + (c + 1) * DIC]
        wgu_gt = np.ascontiguousarray(
            gslice.reshape(HC, 128, DIC // 128, 128).transpose(2, 1, 0, 3)
            .reshape(DIC // 128, 128, HID))
        wgu_ut = np.ascontiguousarray(
            uslice.reshape(HC, 128, DIC // 128, 128).transpose(2, 1, 0, 3)
            .reshape(DIC // 128, 128, HID))
        wd_c = wd_full[c * DIC:(c + 1) * DIC, :]               # [1024, HID]
        wd_t = np.ascontiguousarray(
            wd_c.reshape(DIC // 128, 128, HC, 128).transpose(2, 1, 0, 3)
            .reshape(HC, 128, DIC))
        # [st, p, j*ST+s] <- x.T[c*384 + j*128 + p, st*ST+s]
        xres_c = np.ascontiguousarray(
            xT_bf[c * DMC:(c + 1) * DMC, :].astype(np.float32)
            .reshape(3, 128, NST, ST).transpose(2, 1, 0, 3)
            .reshape(NST, 128, 3 * ST)).astype(bf)
        in_maps.append(dict(
            xnT=xnT, xres=xres_c, wqkv=wqkv_t, wo=wo_t,
            wgu_g=wgu_gt, wgu_u=wgu_ut, wd=wd_t,
            sinT=sinT, cosT=cosT, ident=ident, pmat=pmat, biasp=biasp,
        ))

    res = bass_utils.run_bass_kernel_spmd(nc, in_maps,
                                          core_ids=list(range(NCORES)))
    shards = []
    for c in range(NCORES):
        o = (res.results[c]["mlp_shard"].astype(np.float32)
             + res.results[c]["hs_shard"].astype(np.float32))
        # [st, p, j*ST+s] -> [j*128+p, st*ST+s]
        shards.append(o.reshape(NST, 128, 3, ST).transpose(2, 1, 0, 3)
                      .reshape(DMC, S))
    outT = np.concatenate(shards, axis=0)                      # [HID, S]
    return np.ascontiguousarray(outT.T)[None].astype(np.float32)
